# revision 7
# baseline (speedup 1.0000x reference)
"""HOReID local-features kernel for 8x Trainium2 NeuronCores.

Computes, per sample n (jax reference in reference.py):
  local[k, c] = sum_hw scoremap[n, k, hw] * feat[n, c, hw]   (13 weighted pools)
  glob[c]     = mean_hw feat + max_hw feat                    (GAP + GMP)
  feat_vecs[n] = concat(local, glob)  -> [14, 2048]
  conf[n]     = L1-normalized confidences (host-side glue, 3.5 KB)

Sharding: pure data-parallel, batch N=256 -> 32 samples on each of 8 cores.

Device mapping per sample (chosen via TimelineSim cost-model search):
  - matmul on TensorE: lhsT = saug [HW=128, 14] (col 13 = 1/128 -> GAP row),
    rhs = featT [HW=128, C=2048] host-transposed, 4 N=512 chunks -> PSUM
    [14, 2048] in two bank-pairs, copied to SBUF on ScalarE.
  - GMP (max over hw) needs hw in the free dim; featT has it on partitions.
    Hybrid rebuild of the natural layout: 8 of 16 c-chunks are re-loaded
    from DRAM in natural layout (extra DMA), the other 8 are rebuilt
    on-chip with TensorE fp32 transposes (extra PE); VectorE reduce_max
    over both -> gmax [128, 16]. The 50/50 split balances the DMA vs PE
    rooflines (cost model: 173 us vs 191 us all-transpose / 234 us all-load).
  - gmax -> [16, 128] via TensorE transpose, then an SBUF->SBUF DMA with
    accum_op=add folds it into row 13 (on top of GAP from the 1/128 column).
"""

import sys

if "/opt/trn_rl_repo" not in sys.path:
    sys.path.insert(0, "/opt/trn_rl_repo")

import numpy as np

N_CORES = 8
N, C, H, W, K = 256, 2048, 16, 8, 13
HW = H * W               # 128
S = N // N_CORES         # 32 samples per core
NCHUNK = C // 128        # 16
NAT_CHUNKS = 8           # c-chunks loaded twice (natural layout) for GMP
K14 = K + 1

_PROGRAM = None


def _build_program(reps=1):
    from concourse import bass, bacc, tile

    mybir = bass.mybir
    f32 = mybir.dt.float32

    nc = bacc.Bacc(
        "TRN2",
        target_bir_lowering=False,
        debug=False,
        num_devices=N_CORES,
    )

    featT_d = nc.dram_tensor("featT", [S, HW, C], f32, kind="ExternalInput")
    featN_d = nc.dram_tensor("featN", [S, C, HW], f32, kind="ExternalInput")
    saug_d = nc.dram_tensor("saug", [HW, S, K14], f32, kind="ExternalInput")
    ident_d = nc.dram_tensor("ident", [128, 128], f32, kind="ExternalInput")
    fv_d = nc.dram_tensor("fv", [S, K14, C], f32, kind="ExternalOutput")

    ntr = NCHUNK - NAT_CHUNKS  # chunks rebuilt by PE transpose

    with tile.TileContext(nc) as tc:
        with (
            tc.tile_pool(name="const", bufs=1) as constp,
            tc.tile_pool(name="ft", bufs=4) as ftp,
            tc.tile_pool(name="fn", bufs=3) as fnp,
            tc.tile_pool(name="gmax", bufs=2) as gmaxp,
            tc.tile_pool(name="gtsb", bufs=2) as gtsbp,
            tc.tile_pool(name="osb", bufs=3) as outp,
            tc.tile_pool(name="psO", bufs=2, space="PSUM") as psO,
            tc.tile_pool(name="psT", bufs=2, space="PSUM") as psT,
            tc.tile_pool(name="psG", bufs=1, space="PSUM") as psG,
        ):
            saug = constp.tile([HW, S, K14], f32, tag="saug")
            nc.gpsimd.dma_start(saug[:], saug_d[:])
            ident = constp.tile([128, 128], f32, tag="ident")
            nc.gpsimd.dma_start(ident[:], ident_d[:])

            for i in [i for _ in range(reps) for i in range(S)]:
                # transposed layout feeds the matmuls (8 KB/partition contig)
                ft = ftp.tile([HW, C], f32, tag="ft")
                nc.sync.dma_start(ft[:], featT_d[i])

                gmax = gmaxp.tile([128, NCHUNK], f32, tag="gmax")
                outsb = outp.tile([K14, C], f32, tag="outsb")

                # 13 weighted pools + GAP in one matmul group
                for h in range(2):
                    ps = psO.tile([K14, 1024], f32, tag="ps")
                    for j in range(2):
                        col = (h * 2 + j) * 512
                        nc.tensor.matmul(
                            ps[:, j * 512 : (j + 1) * 512],
                            saug[:, i, :],
                            ft[:, col : col + 512],
                            start=True,
                            stop=True,
                        )
                    nc.scalar.copy(outsb[:, h * 1024 : (h + 1) * 1024], ps[:])

                # GMP: natural-layout chunks (half re-loaded, half transposed)
                fn = fnp.tile([128, NAT_CHUNKS, HW], f32, tag="fn")
                nc.sync.dma_start(
                    fn[:],
                    featN_d[i, ntr * 128 :].rearrange(
                        "(ch cin) hw -> cin ch hw", cin=128
                    ),
                )
                nc.vector.tensor_reduce(
                    gmax[:, ntr:],
                    fn[:],
                    axis=mybir.AxisListType.X,
                    op=mybir.AluOpType.max,
                )
                for r in range(ntr // 4):
                    pst = psT.tile([128, 4, HW], f32, tag="pst")
                    for j in range(4):
                        ch = r * 4 + j
                        nc.tensor.transpose(
                            pst[:, j, :], ft[:, ch * 128 : (ch + 1) * 128], ident[:]
                        )
                    nc.vector.tensor_reduce(
                        gmax[:, r * 4 : (r + 1) * 4],
                        pst[:],
                        axis=mybir.AxisListType.X,
                        op=mybir.AluOpType.max,
                    )

                # gmax [128, 16] -> [16, 128] so it can be laid along row 13
                gt = psG.tile([NCHUNK, 128], f32, tag="gt")
                nc.tensor.transpose(gt[:], gmax[:], ident[:])
                gtsb = gtsbp.tile([NCHUNK, 128], f32, tag="gtsb")
                nc.vector.tensor_copy(gtsb[:], gt[:])

                # row 13 += gmax (GAP already there from the 1/128 column)
                nc.gpsimd.dma_start(
                    outsb[K : K + 1, :].rearrange("p (a b) -> p a b", a=NCHUNK),
                    gtsb[:],
                    accum_op=mybir.AluOpType.add,
                )

                nc.scalar.dma_start(fv_d[i], outsb[:])

    nc.compile()
    return nc


def _get_program():
    global _PROGRAM
    if _PROGRAM is None:
        _PROGRAM = _build_program()
    return _PROGRAM


def kernel(feat, scoremap, keypoints_confidence, trace=False, **_ignored):
    from concourse.bass_utils import run_bass_kernel_spmd

    feat = np.ascontiguousarray(np.asarray(feat), dtype=np.float32)
    scoremap = np.ascontiguousarray(np.asarray(scoremap), dtype=np.float32)
    kc = np.asarray(keypoints_confidence, dtype=np.float32)

    featf = feat.reshape(N, C, HW)
    featT = np.ascontiguousarray(featf.transpose(0, 2, 1))  # [N, HW, C]
    sa = scoremap.reshape(N, K, HW).transpose(2, 0, 1)      # [HW, N, K]
    saug = np.concatenate(
        [sa, np.full((HW, N, 1), 1.0 / HW, np.float32)], axis=2
    )                                                        # [HW, N, 14]
    ident = np.eye(128, dtype=np.float32)

    in_maps = []
    for k in range(N_CORES):
        sl = slice(k * S, (k + 1) * S)
        in_maps.append(
            {
                "featT": featT[sl],
                "featN": np.ascontiguousarray(featf[sl]),
                "saug": np.ascontiguousarray(saug[:, sl, :]),
                "ident": ident,
            }
        )

    nc = _get_program()
    res = run_bass_kernel_spmd(nc, in_maps, list(range(N_CORES)), trace=trace)
    kernel.last_results = res

    fv = np.concatenate(
        [res.results[k]["fv"] for k in range(N_CORES)], axis=0
    )  # [256, 14, 2048]

    # confidence head: tiny (256x14) L1 normalization, host glue
    conf = np.concatenate([kc, np.ones((N, 1), kc.dtype)], axis=1)
    head = conf[:, :K] / np.maximum(
        np.abs(conf[:, :K]).sum(axis=1, keepdims=True), 1e-12
    )
    tail = conf[:, K:] / np.maximum(
        np.abs(conf[:, K:]).sum(axis=1, keepdims=True), 1e-12
    )
    conf = np.concatenate([head, tail], axis=1).astype(np.float32)

    return fv, conf
